# revision 6
# baseline (speedup 1.0000x reference)
# GAT (2-layer, PyG-style with self-loops) on 8 Trainium2 NeuronCores.
#
# Sharding: destination nodes are partitioned across the 8 cores (12500
# each). Every core redundantly computes the layer-1 linear projection
# for all N nodes (cheaper than all-gathering it), so layer-1 message
# gathers are purely local. Layer-2 projections are computed only for
# owned nodes and exchanged with a single AllGather.
#
# Per-edge work uses the SWDGE dma_gather (int16 indices => the node
# table is gathered in 4 chunks of 25000 rows, edges sorted by source
# chunk). Destination-segmented softmax/aggregation is done WITHOUT
# dma_scatter_add (its CCE read-modify-write races on duplicate
# indices): edges are ordered by (src_chunk, dst_tile) and aggregated
# with PE matmuls against host-built one-hot selector matrices,
# accumulating per-dst-tile partials in PSUM and an SBUF accumulator.
# exp() is applied without the segment-max shift (logits are bounded,
# fp32 exp cannot overflow; the softmax ratio is unchanged), scaled by
# 2^-8 so fp16 messages cannot overflow; the scale cancels in the
# normalization.
import math
import numpy as np

N = 100000
IN_CH = 128
HID = 32
HEADS = 4
OUT_CH = 64
NEG_SLOPE = 0.2
EPS = 1e-16
NCORES = 8
LN2x8 = 8.0 * math.log(2.0)


class Cfg:
    def __init__(self, n, e, tmax=4096):
        assert n % (NCORES * 2) == 0
        self.N = n
        self.E = e
        self.NLOC = n // NCORES
        self.NCHUNK = 4
        self.CHUNK = n // 4
        self.DT = (self.NLOC + 127) // 128
        self.LAST_ROWS = self.NLOC - 128 * (self.DT - 1)
        self.TMAX = tmax


FULL = Cfg(N, 1600000, tmax=4096)


# ---------------------------------------------------------------- host prep
def _prep_edges(cfg, src, dst):
    """Order each core's edges by (src_chunk, dst_tile), pad every
    (chunk, dst_tile) segment to a common multiple-of-128 size across
    cores, and chop each chunk's segment list into gather tiles.

    Returns (plan, per_core) where plan is the shared program structure
    and per_core holds the idx / selector arrays."""
    nloc, chunk, dt_n = cfg.NLOC, cfg.CHUNK, cfg.DT
    core = dst // nloc
    per_core_edges = []
    counts = np.zeros((NCORES, cfg.NCHUNK, dt_n), np.int64)
    for c in range(NCORES):
        m = core == c
        s, d = src[m], dst[m]
        dl = d - c * nloc
        k = s // chunk
        dt = dl // 128
        order = np.lexsort((dl, dt, k))
        s, dl, k, dt = s[order], dl[order], k[order], dt[order]
        np.add.at(counts[c], (k, dt), 1)
        per_core_edges.append((s, dl, k, dt))

    seg = counts.max(axis=0)                    # [NCHUNK, DT]
    seg = ((seg + 127) // 128) * 128            # pad to 128-multiples

    # tiles: greedy pack whole segments up to TMAX edges
    tiles = []                                  # (k, n_edges, [(dt, ngroups)...])
    for k in range(cfg.NCHUNK):
        cur, curn = [], 0
        for dt in range(dt_n):
            n = int(seg[k, dt])
            if n == 0:
                continue
            if curn + n > cfg.TMAX and curn > 0:
                tiles.append((k, curn, cur))
                cur, curn = [], 0
            cur.append((dt, n // 128))
            curn += n
        if curn:
            tiles.append((k, curn, cur))
    etot = sum(t[1] for t in tiles)

    # per-core arrays in the padded order
    per_core = []
    for c in range(NCORES):
        s, dl, k, dt = per_core_edges[c]
        isrc = np.zeros(etot, np.int16)
        idst = np.zeros(etot, np.int16)
        dstoff = np.zeros(etot, np.int16)
        valid = np.zeros(etot, bool)
        # segment start offsets in padded layout, per (k, dt)
        seg_off = np.zeros((cfg.NCHUNK, dt_n), np.int64)
        off = 0
        for (tk, tn, segs) in tiles:
            o = off
            for (sdt, ng) in segs:
                seg_off[tk, sdt] = o
                o += ng * 128
            off += tn
        # place this core's edges at the head of each padded segment
        pos_in_seg = np.zeros_like(s)
        lin = k * dt_n + dt
        order_stable = np.argsort(lin, kind="stable")
        lin_sorted = lin[order_stable]
        first = np.searchsorted(lin_sorted, lin_sorted)
        pos_in_seg[order_stable] = np.arange(len(s)) - first
        pos = seg_off[k, dt] + pos_in_seg
        isrc[pos] = (s - k * chunk).astype(np.int16)
        idst[pos] = dl.astype(np.int16)
        dstoff[pos] = (dl - dt * 128).astype(np.int16)
        valid[pos] = True
        sel = np.zeros((etot, 128), np.float16)
        sel[np.arange(etot)[valid], dstoff[valid]] = 1.0
        per_core.append({"isrc": isrc, "idst": idst, "sel": sel})

    plan = {"tiles": tiles, "etot": etot}
    return plan, per_core


def _wrap16(idx):
    """int16 idx array -> SWDGE [128, n/16] wrapped layout."""
    assert idx.shape[0] % 16 == 0
    return np.tile(idx.reshape(-1, 16).T, (8, 1)).copy()


def _host_inputs(cfg, x, src, dst, W1, as1, ad1, b1, W2, as2, ad2, b2):
    plan, per_core = _prep_edges(cfg, src, dst)
    heads, hid = (HEADS, HID) if W1.shape[1] == HEADS * HID else (W1.shape[1] // HID, HID)
    v1s = np.stack([W1[:, i * hid:(i + 1) * hid] @ as1[0, i] for i in range(heads)], axis=1)
    v1d = np.stack([W1[:, i * hid:(i + 1) * hid] @ ad1[0, i] for i in range(heads)], axis=1)
    w1cat = np.concatenate([W1, v1s, v1d], axis=1).astype(np.float32)      # [128,136]
    v2s = (W2 @ as2[0, 0])[:, None]
    v2d = (W2 @ ad2[0, 0])[:, None]
    w2cat = np.concatenate([W2, v2s, v2d], axis=1).astype(np.float32)      # [128,66]
    shared = {
        "xT": np.ascontiguousarray(x.T.astype(np.float32)),
        "w1cat": w1cat,
        "w2cat": w2cat,
        "b1rep": np.tile(b1.astype(np.float32)[None, :], (128, 1)),
        "b2rep": np.tile(b2.astype(np.float32)[None, :], (128, 1)),
        "ident": np.eye(128, dtype=np.float32),
    }
    in_maps = []
    for c in range(NCORES):
        m = dict(shared)
        m["isrc"] = _wrap16(per_core[c]["isrc"])
        m["idst"] = _wrap16(per_core[c]["idst"])
        m["sel"] = per_core[c]["sel"]
        in_maps.append(m)
    return plan, in_maps


# ---------------------------------------------------------------- program
def _build_program(cfg, plan):
    import concourse.bacc as bacc
    import concourse.bass as bass
    import concourse.mybir as mybir
    import concourse.tile as tile
    from concourse import library_config

    f32, f16, i16 = mybir.dt.float32, mybir.dt.float16, mybir.dt.int16
    AF = mybir.ActivationFunctionType
    n, nloc, chunk, dt_n = cfg.N, cfg.NLOC, cfg.CHUNK, cfg.DT
    etot = plan["etot"]
    tiles = plan["tiles"]

    nc = bacc.Bacc("TRN2", target_bir_lowering=False)
    xT = nc.declare_dram_parameter("xT", [128, n], f32, isOutput=False)
    w1cat = nc.declare_dram_parameter("w1cat", [128, 136], f32, isOutput=False)
    w2cat = nc.declare_dram_parameter("w2cat", [128, 66], f32, isOutput=False)
    b1rep = nc.declare_dram_parameter("b1rep", [128, 128], f32, isOutput=False)
    b2rep = nc.declare_dram_parameter("b2rep", [128, 64], f32, isOutput=False)
    ident = nc.declare_dram_parameter("ident", [128, 128], f32, isOutput=False)
    isrc = nc.declare_dram_parameter("isrc", [128, etot // 16], i16, isOutput=False)
    idst = nc.declare_dram_parameter("idst", [128, etot // 16], i16, isOutput=False)
    seld = nc.declare_dram_parameter("sel", [etot, 128], f16, isOutput=False)
    outp = nc.declare_dram_parameter("out", [nloc, 64], f32, isOutput=True)

    T1 = nc.dram_tensor("T1", [n, 256], f16)
    T2loc = nc.dram_tensor("T2loc", [nloc, 128], f16)
    T2full = nc.dram_tensor("T2full", [n, 128], f16, addr_space="Shared")
    LS1 = nc.dram_tensor("LS1", [nloc, 64], f32)
    LS2 = nc.dram_tensor("LS2", [nloc, 64], f32)

    sel3 = seld.reshape([etot // 128, 128, 128]).rearrange("g p d -> p g d")

    ntile_a = (n + 127) // 128

    with tile.TileContext(nc) as tc:
        nc.gpsimd.load_library(library_config.mlp)
        with tc.tile_pool(name="const", bufs=1) as pc0:
            w1c = pc0.tile([128, 136], f32)
            nc.sync.dma_start(out=w1c[:], in_=w1cat[:])
            w2c = pc0.tile([128, 66], f32)
            nc.sync.dma_start(out=w2c[:], in_=w2cat[:])
            b1s = pc0.tile([128, 128], f32)
            nc.sync.dma_start(out=b1s[:], in_=b1rep[:])
            b2s = pc0.tile([128, 64], f32)
            nc.sync.dma_start(out=b2s[:], in_=b2rep[:])
            ids = pc0.tile([128, 128], f32)
            nc.sync.dma_start(out=ids[:], in_=ident[:])
            ebias = pc0.tile([128, 1], f32)
            nc.vector.memset(ebias[:], -LN2x8)

            # ---- stage A: full linear-1 table  T1[n] = [h1 f16 128 | a_src,a_dst f32 8]
            with tc.tile_pool(name="stA", bufs=3) as pa, \
                 tc.tile_pool(name="psA", bufs=8, space="PSUM") as ppa:
                for i in range(ntile_a):
                    n0 = i * 128
                    nn = min(128, n - n0)
                    xt = pa.tile([128, 128], f32, tag="xt")
                    nc.sync.dma_start(out=xt[:, :nn], in_=xT[:, n0:n0 + nn])
                    ps = ppa.tile([128, 136], f32, tag="psA")
                    nc.tensor.matmul(ps[:nn, :], xt[:, :nn], w1c[:], start=True, stop=True)
                    t1r = pa.tile([128, 256], f16, tag="t1r")
                    nc.vector.tensor_copy(t1r[:nn, 0:128], ps[:nn, 0:128])
                    nc.vector.tensor_copy(t1r.bitcast(f32)[:nn, 64:72], ps[:nn, 128:136])
                    nc.gpsimd.dma_start(out=T1[n0:n0 + nn, 0:144], in_=t1r[:nn, 0:144])

            # local side table for the dst-side gather
            pid = nc.gpsimd.partition_id()
            nc.gpsimd.dma_start(out=LS1[:, 0:8],
                                in_=T1.bitcast(f32)[bass.ds(pid * nloc, nloc), 64:72])

            # ---- edge pass helper
            def edge_pass(acc, tbl, ls, heads, ch, cm, aoff):
                el = tbl.shape[1]
                tview = tbl
                off = 0
                for (k, tn, segs) in tiles:
                    tg = tn // 128
                    ist = pi.tile([128, tn // 16], i16, tag="ist")
                    nc.gpsimd.dma_start(out=ist[:], in_=isrc[:, off // 16:(off + tn) // 16])
                    idt = pi.tile([128, tn // 16], i16, tag="idt")
                    nc.gpsimd.dma_start(out=idt[:], in_=idst[:, off // 16:(off + tn) // 16])
                    G = pg.tile([128, tg, el], f16, tag="G")
                    nc.gpsimd.dma_gather(G[:], tview[k * chunk:(k + 1) * chunk, :],
                                         ist[:], tn, tn, el)
                    D = pg.tile([128, tg, 64], f32, tag="D")
                    nc.gpsimd.dma_gather(D[:], ls[:, :], idt[:], tn, tn, 64)
                    SelT = pg.tile([128, tg, 128], f16, tag="Sel")
                    nc.sync.dma_start(out=SelT[:], in_=sel3[:, off // 128:(off + tn) // 128, :])
                    asrc = G[:, :, ch:ch + 2 * heads].bitcast(f32)
                    e = pw.tile([128, tg, heads], f32, tag="e")
                    nc.vector.tensor_add(e[:], asrc, D[:, :, aoff:aoff + heads])
                    nc.vector.scalar_tensor_tensor(e[:], e[:], NEG_SLOPE, e[:],
                                   mybir.AluOpType.mult, mybir.AluOpType.max)
                    w = pw.tile([128, tg, heads], f32, tag="w")
                    nc.scalar.activation(w[:], e[:], AF.Exp, bias=ebias[:, 0:1])
                    M = pg.tile([128, tg, cm], f16, tag="M")
                    hw = ch // heads
                    for h in range(heads):
                        nc.vector.tensor_mul(
                            M[:, :, h * hw:(h + 1) * hw],
                            G[:, :, h * hw:(h + 1) * hw],
                            w[:, :, h:h + 1].broadcast_to([128, tg, hw]))
                    nc.vector.tensor_copy(M[:, :, ch:ch + heads], w[:])
                    gl = 0
                    for (dt, ng) in segs:
                        psm = ppe.tile([128, cm], f32, tag="psE")
                        for j in range(ng):
                            nc.tensor.matmul(psm[:], SelT[:, gl + j, :], M[:, gl + j, :],
                                             start=(j == 0), stop=(j == ng - 1))
                        nc.vector.tensor_add(acc[:, dt, :], acc[:, dt, :], psm[:])
                        gl += ng
                    off += tn

            # ---- layer 1 edges + stage C
            with tc.tile_pool(name="acc1", bufs=1) as pacc:
                ACC1 = pacc.tile([128, dt_n, 132], f32)
                nc.vector.memset(ACC1[:], 0.0)
                with tc.tile_pool(name="pe1", bufs=2) as pg, \
                     tc.tile_pool(name="pi1", bufs=2) as pi, \
                     tc.tile_pool(name="pw1", bufs=2) as pw, \
                     tc.tile_pool(name="ppe1", bufs=8, space="PSUM") as ppe:
                    edge_pass(ACC1, T1, LS1, HEADS, 128, 132, 4)

                with tc.tile_pool(name="stC", bufs=3) as pcs, \
                     tc.tile_pool(name="psC", bufs=4, space="PSUM") as ppc:
                    for dt in range(dt_n):
                        rows = 128 if dt < dt_n - 1 else cfg.LAST_ROWS
                        winv = pcs.tile([128, HEADS], f32, tag="winv")
                        nc.vector.tensor_scalar_add(winv[:], ACC1[:, dt, 128:132], EPS)
                        nc.vector.reciprocal(winv[:], winv[:])
                        h = pcs.tile([128, 128], f32, tag="h")
                        for hh in range(HEADS):
                            nc.vector.tensor_mul(h[:, hh * 32:(hh + 1) * 32],
                                                 ACC1[:, dt, hh * 32:(hh + 1) * 32],
                                                 winv[:, hh:hh + 1].broadcast_to([128, 32]))
                        nc.vector.tensor_add(h[:], h[:], b1s[:])
                        # elu
                        r = pcs.tile([128, 128], f32, tag="r")
                        nc.scalar.activation(r[:], h[:], AF.Relu)
                        mm = pcs.tile([128, 128], f32, tag="mm")
                        nc.vector.tensor_scalar_min(mm[:], h[:], 0.0)
                        nc.scalar.activation(mm[:], mm[:], AF.Exp)
                        nc.vector.tensor_add(r[:], r[:], mm[:])
                        nc.vector.tensor_scalar_add(r[:], r[:], -1.0)
                        pst = ppc.tile([128, 128], f32, tag="psT")
                        nc.tensor.transpose(pst[:], r[:], ids[:])
                        hT = pcs.tile([128, 128], f32, tag="hT")
                        nc.vector.tensor_copy(hT[:], pst[:])
                        ps2 = ppc.tile([128, 66], f32, tag="ps2")
                        nc.tensor.matmul(ps2[:rows, :], hT[:, :rows], w2c[:],
                                         start=True, stop=True)
                        t2r = pcs.tile([128, 128], f16, tag="t2r")
                        nc.vector.tensor_copy(t2r[:rows, 0:64], ps2[:rows, 0:64])
                        nc.vector.tensor_copy(t2r.bitcast(f32)[:rows, 32:34],
                                              ps2[:rows, 64:66])
                        n0 = dt * 128
                        nc.gpsimd.dma_start(out=T2loc[n0:n0 + rows, 0:68], in_=t2r[:rows, 0:68])
                        nc.gpsimd.dma_start(out=LS2[n0:n0 + rows, 0:2],
                                            in_=t2r.bitcast(f32)[:rows, 32:34])

            # ---- exchange layer-2 projections
            nc.gpsimd.collective_compute(
                "AllGather", mybir.AluOpType.bypass,
                replica_groups=[list(range(NCORES))],
                ins=[T2loc.reshape([nloc * 128])[:]],
                outs=[T2full.reshape([n * 128])[:]],
            )

            # ---- layer 2 edges + output
            with tc.tile_pool(name="acc2", bufs=1) as pacc2:
                ACC2 = pacc2.tile([128, dt_n, 65], f32)
                nc.vector.memset(ACC2[:], 0.0)
                with tc.tile_pool(name="pe2", bufs=2) as pg, \
                     tc.tile_pool(name="pi2", bufs=2) as pi, \
                     tc.tile_pool(name="pw2", bufs=2) as pw, \
                     tc.tile_pool(name="ppe2", bufs=8, space="PSUM") as ppe:
                    edge_pass(ACC2, T2full, LS2, 1, 64, 65, 1)

                with tc.tile_pool(name="stF", bufs=3) as pf:
                    for dt in range(dt_n):
                        rows = 128 if dt < dt_n - 1 else cfg.LAST_ROWS
                        winv2 = pf.tile([128, 1], f32, tag="winv2")
                        nc.vector.tensor_scalar_add(winv2[:], ACC2[:, dt, 64:65], EPS)
                        nc.vector.reciprocal(winv2[:], winv2[:])
                        o = pf.tile([128, 64], f32, tag="o")
                        nc.vector.tensor_scalar_mul(o[:], ACC2[:, dt, 0:64], winv2[:, 0:1])
                        nc.vector.tensor_add(o[:], o[:], b2s[:])
                        n0 = dt * 128
                        nc.sync.dma_start(out=outp[n0:n0 + rows, :], in_=o[:rows, :])
    nc.compile()
    return nc


# ---------------------------------------------------------------- entry
_CACHE = {}


def prepare(cfg, x, edge_index, W1, att_src1, att_dst1, b1, W2, att_src2, att_dst2, b2):
    n = cfg.N
    src = np.concatenate([np.asarray(edge_index[0]), np.arange(n)]).astype(np.int64)
    dst = np.concatenate([np.asarray(edge_index[1]), np.arange(n)]).astype(np.int64)
    plan, in_maps = _host_inputs(cfg, np.asarray(x, np.float32), src, dst,
                                 np.asarray(W1), np.asarray(att_src1), np.asarray(att_dst1),
                                 np.asarray(b1), np.asarray(W2), np.asarray(att_src2),
                                 np.asarray(att_dst2), np.asarray(b2))
    key = (cfg.N, cfg.E, cfg.TMAX, plan["etot"], tuple((k, t) for k, t, _ in plan["tiles"]))
    if key in _CACHE:
        nc = _CACHE[key]
    else:
        nc = _build_program(cfg, plan)
        _CACHE.clear()
        _CACHE[key] = nc
    return nc, in_maps, plan


def _run(cfg, x, edge_index, W1, att_src1, att_dst1, b1, W2, att_src2, att_dst2, b2,
         sim=False):
    nc, in_maps, plan = prepare(cfg, x, edge_index, W1, att_src1, att_dst1, b1,
                                W2, att_src2, att_dst2, b2)
    if sim:
        import concourse.bass_interp as bass_interp
        s = bass_interp.MultiCoreSim(nc, NCORES, require_finite=False, require_nnan=False)
        for c in range(NCORES):
            for k, v in in_maps[c].items():
                s.cores[c].tensor(k)[:] = v
        s.simulate()
        outs = [np.array(s.cores[c].tensor("out")) for c in range(NCORES)]
    else:
        from concourse.bass_utils import run_bass_kernel_spmd
        res = run_bass_kernel_spmd(nc, in_maps, list(range(NCORES))).results
        outs = [res[c]["out"] for c in range(NCORES)]
    return np.concatenate(outs, axis=0)


def kernel(x, edge_index, W1, att_src1, att_dst1, b1, W2, att_src2, att_dst2, b2):
    return _run(FULL, x, edge_index, W1, att_src1, att_dst1, b1,
                W2, att_src2, att_dst2, b2, sim=False)
